# revision 11
# baseline (speedup 1.0000x reference)
"""Trainium2 Bass kernel for nn_DecentralizedCoordinator (GNN message passing).

Strategy (8 NeuronCores, SPMD), v3:
- Nodes sharded by id: core k owns 98 blocks of 128 dst nodes (12544/core).
- Edges partitioned by destination (core, block); per-block slot streams are
  column-aligned (multiples of 128). Host materializes the halo / edge-source
  feature table ET[k] = x[src] in slot order (bf16, partition-major layout)
  — the "all-gather + edge partition" sharding step — so the device streams
  it SEQUENTIALLY at line rate instead of descriptor-bound random gathers.
- Device L1 (all value arithmetic):
  logits = x·w_lead + b_lead per owned node (vector),
  scaled one-hots ohs[slot,d] = (dstl[slot]==d) * recip[d] built on
  DVE+ACT (scalar_tensor_tensor is_equal*mult, recip row broadcast across
  partitions by gpsimd), segment mean directly transposed via
  matmul(lhsT=ET_col, rhs=ohs_col) accumulating meanT in PSUM,
  MLP transposed: w1 -> gelu(+b1) -> w2 (+b2 fused in PSUM copy) ->
  reportsT bf16, staged DMA out (feature-major).
- Host between launches: assemble per-dst padded logits layout (epad/srcp1)
  from L1 logits — pure indexed reshuffle, pattern known at preprocess.
- Device L2: leader election (reduce_max / is_equal / mult / reduce_max,
  exact reference tie-break), then the value-dependent gather
  reports[leader] via batched indirect DMA (per-partition offsets).

Host only shards/permutes/reshuffles by precomputed indices; every operation
on runtime values (sums, means, MLP, comparisons, final gather) is on device.
"""
import hashlib
import sys

import numpy as np
import ml_dtypes

sys.path.insert(0, "/opt/trn_rl_repo")

import concourse.bass as bass
import concourse.tile as tile
from concourse import bacc, mybir
from concourse.bass_utils import run_bass_kernel_spmd

dt = mybir.dt
bf16 = ml_dtypes.bfloat16

P = 128
NCORES = 8
BPC = 98                 # dst blocks per core
NPC = BPC * P            # 12544 nodes per core
NPAD = NCORES * NPC      # 100352 padded node count
N_NODES = 100000
H = 128
C = 128
CW = 64                  # ET window width (columns of 128 slots)
NEG = -3.0e38

CORES = list(range(NCORES))


def _assign_nodes(col, n_nodes):
    """Balanced node -> (core, block, slot): equalize per-block in-degree sums
    across cores. Returns node2kbp [N,3] and inv [NCORES,BPC,P]."""
    indeg = np.bincount(col, minlength=n_nodes)
    order = np.argsort(-indeg, kind="stable")
    node2kbp = np.zeros((n_nodes, 3), np.int64)
    inv = np.full((NCORES, BPC, P), -1, np.int64)
    BIG = 1 << 40
    for b in range(BPC):
        sl = order[b * NCORES * P: (b + 1) * NCORES * P]
        loads = np.zeros(NCORES, np.int64)
        caps = np.full(NCORES, P, np.int64)
        slots = np.zeros(NCORES, np.int64)
        for nd in sl:
            cost = loads.astype(np.float64).copy()
            cost[caps == 0] = BIG
            kbest = int(np.argmin(cost))
            loads[kbest] += indeg[nd]
            caps[kbest] -= 1
            p = int(slots[kbest])
            slots[kbest] += 1
            node2kbp[nd] = (kbest, b, p)
            inv[kbest, b, p] = nd
    return node2kbp, inv


def _preprocess(edge_index):
    row = np.asarray(edge_index[0], np.int64)
    col = np.asarray(edge_index[1], np.int64)

    node2kbp, inv = _assign_nodes(col, N_NODES)
    core = node2kbp[col, 0]
    blk = node2kbp[col, 1]
    dstl = node2kbp[col, 2]

    gkey = core * BPC + blk
    order = np.argsort(gkey, kind="stable")
    src_s = row[order]
    dstl_s = dstl[order]
    counts = np.bincount(gkey, minlength=NCORES * BPC).reshape(NCORES, BPC)
    starts = np.concatenate(
        [[0], np.cumsum(counts.reshape(-1))[:-1]]).reshape(counts.shape)

    m_b = counts.max(axis=0)                       # [BPC]
    mcols = (m_b + P - 1) // P                     # columns per block
    colbase = np.concatenate([[0], np.cumsum(mcols)[:-1]])
    NCOL = int(mcols.sum())
    occ_block = np.zeros(NCOL, np.int64)
    for b in range(BPC):
        occ_block[colbase[b]: colbase[b] + mcols[b]] = b

    # per-core slot source ids + per-occurrence dst-slot rows
    srcidx = np.full((NCORES, NCOL * P), N_NODES, np.int64)
    dstl_occ = np.full((NCORES, P, NCOL), -1.0, np.float32)
    for k in range(NCORES):
        for b in range(BPC):
            n = int(counts[k, b])
            if n == 0:
                continue
            s0 = int(starts[k, b])
            g0 = int(colbase[b]) * P
            srcidx[k, g0: g0 + n] = src_s[s0: s0 + n]
            pos = np.arange(n)
            cc = int(colbase[b]) + pos // P
            dstl_occ[k, pos % P, cc] = dstl_s[s0: s0 + n]
    dstl_bf = dstl_occ.astype(bf16)

    # recip of true in-degree per owned node, flat [NCORES, 1, NPC] (b,d)
    indeg = np.bincount(col, minlength=N_NODES).astype(np.float32)
    cnt = np.where(inv >= 0, indeg[np.maximum(inv, 0)], 0.0)  # [NC,BPC,P]
    recip_flat = (1.0 / np.maximum(cnt, 1.0)).reshape(
        NCORES, 1, NPC).astype(bf16)
    recip_flat = np.ascontiguousarray(recip_flat)

    # leader-election padded layout (extended edges with self loops)
    deg = np.bincount(col, minlength=NPAD) + 1
    WU = int(deg.max())
    elog_src = np.full((NCORES, P, BPC, WU), -1, np.int64)
    dorder = np.argsort(col, kind="stable")
    row_d = row[dorder]
    dst_starts = np.concatenate(
        [[0], np.cumsum(np.bincount(col, minlength=NPAD))])
    for k in range(NCORES):
        for b in range(BPC):
            for p in range(P):
                d = int(inv[k, b, p])
                if d < 0:
                    continue
                s0, s1 = int(dst_starts[d]), int(dst_starts[d + 1])
                m = s1 - s0
                elog_src[k, p, b, 0] = d
                if m > 0:
                    elog_src[k, p, b, 1: 1 + m] = row_d[s0:s1]
    elog_src = elog_src.reshape(NCORES, P, BPC * WU)
    srcp1 = np.where(elog_src >= 0, elog_src + 1, 0).astype(np.float32)

    return dict(
        NCOL=NCOL, mcols=mcols, colbase=colbase, occ_block=occ_block,
        srcidx=srcidx, dstl_bf=dstl_bf, recip_flat=recip_flat,
        WU=WU, elog_src=elog_src, srcp1=srcp1,
        node2kbp=node2kbp, inv=inv,
    )


# ---------------------------------------------------------------------------
# launch 1: logits + segment mean + MLP -> reportsT (bf16, feature-major)
# ---------------------------------------------------------------------------

def _build_l1(pp):
    NCOL = pp["NCOL"]
    mcols = pp["mcols"]
    colbase = pp["colbase"]
    occ_block = pp["occ_block"]

    nc = bacc.Bacc("TRN2", target_bir_lowering=False, debug=False,
                   num_devices=NCORES)
    et_d = nc.dram_tensor("et", [P, NCOL * H], dt.bfloat16,
                          kind="ExternalInput")
    xf_d = nc.dram_tensor("xf2", [P, BPC * H], dt.float32,
                          kind="ExternalInput")
    dstl_d = nc.dram_tensor("dstl", [P, NCOL], dt.bfloat16,
                            kind="ExternalInput")
    recipf_d = nc.dram_tensor("recipf", [1, NPC], dt.bfloat16,
                              kind="ExternalInput")
    wrep_d = nc.dram_tensor("wrep", [P, H], dt.float32, kind="ExternalInput")
    blead_d = nc.dram_tensor("blead", [P, 1], dt.float32,
                             kind="ExternalInput")
    w1_d = nc.dram_tensor("w1", [H, H], dt.bfloat16, kind="ExternalInput")
    b1_d = nc.dram_tensor("b1", [P, 1], dt.float32, kind="ExternalInput")
    w2_d = nc.dram_tensor("w2", [H, C], dt.bfloat16, kind="ExternalInput")
    b2_d = nc.dram_tensor("b2c", [C, 1], dt.float32, kind="ExternalInput")

    logits_o = nc.dram_tensor("logits_o", [P, BPC], dt.float32,
                              kind="ExternalOutput")
    reports_o = nc.dram_tensor("reports_o", [P, NPC], dt.bfloat16,
                               kind="ExternalOutput")

    with tile.TileContext(nc) as tc:
        with (
            tc.tile_pool(name="const", bufs=1) as cp,
            tc.tile_pool(name="xf", bufs=2) as xp,
            tc.tile_pool(name="g", bufs=3) as gp,
            tc.tile_pool(name="oh", bufs=3) as op_,
            tc.tile_pool(name="small", bufs=3) as sp,
            tc.tile_pool(name="mstage", bufs=2) as mp,
            tc.tile_pool(name="stage", bufs=2) as stp,
            tc.tile_pool(name="sums_ps", bufs=3, space="PSUM") as sums_pp,
            tc.tile_pool(name="hpre_ps", bufs=2, space="PSUM") as hpre_pp,
            tc.tile_pool(name="rep_ps", bufs=2, space="PSUM") as rep_pp,
        ):
            dstl_t = cp.tile([P, NCOL], dt.bfloat16)
            nc.sync.dma_start(dstl_t[:], dstl_d[:, :])
            recipf_t = cp.tile([1, NPC], dt.bfloat16)
            nc.sync.dma_start(recipf_t[:], recipf_d[:, :])
            wrep_t = cp.tile([P, H], dt.float32)
            nc.sync.dma_start(wrep_t[:], wrep_d[:, :])
            blead_t = cp.tile([P, 1], dt.float32)
            nc.sync.dma_start(blead_t[:], blead_d[:, :])
            w1_t = cp.tile([H, H], dt.bfloat16)
            nc.sync.dma_start(w1_t[:], w1_d[:, :])
            b1_t = cp.tile([P, 1], dt.float32)
            nc.sync.dma_start(b1_t[:], b1_d[:, :])
            w2_t = cp.tile([H, C], dt.bfloat16)
            nc.sync.dma_start(w2_t[:], w2_d[:, :])
            b2_t = cp.tile([C, 1], dt.float32)
            nc.sync.dma_start(b2_t[:], b2_d[:, :])

            iota_i = cp.tile([P, P], dt.int32)
            nc.gpsimd.iota(iota_i[:], pattern=[[1, P]], base=0,
                           channel_multiplier=0)
            iota_f = cp.tile([P, P], dt.bfloat16)
            nc.vector.tensor_copy(iota_f[:], iota_i[:])

            # recip rows broadcast across partitions: recipR[p, b*128+d]
            recipR = cp.tile([P, NPC], dt.bfloat16)
            RBC = 1792
            for r0 in range(0, NPC, RBC):
                nc.gpsimd.partition_broadcast(
                    recipR[:, r0: r0 + RBC], recipf_t[0:1, r0: r0 + RBC])

            # ---- logits (xf2 partition-major: [p, b*H+f]) ------------------
            logits_sb = cp.tile([P, BPC], dt.float32)
            XC = 14
            for ch in range((BPC + XC - 1) // XC):
                b0 = ch * XC
                nb = min(XC, BPC - b0)
                xfc = xp.tile([P, XC * H], dt.float32, tag="xfc")
                nc.sync.dma_start(xfc[:, : nb * H],
                                  xf_d[:, b0 * H: (b0 + nb) * H])
                for j in range(nb):
                    tmp = xp.tile([P, H], dt.float32, tag="ltmp")
                    nc.vector.tensor_tensor(
                        out=tmp[:], in0=xfc[:, j * H: (j + 1) * H],
                        in1=wrep_t[:], op=mybir.AluOpType.mult)
                    nc.vector.reduce_sum(
                        out=logits_sb[:, b0 + j: b0 + j + 1], in_=tmp[:],
                        axis=mybir.AxisListType.X)
            logits_out = sp.tile([P, BPC], dt.float32, tag="lgout")
            nc.vector.tensor_scalar_add(logits_out[:], logits_sb[:],
                                        blead_t[:, :1])
            nc.sync.dma_start(logits_o[:, :], logits_out[:])

            # ---- ET windows + scaled one-hots ------------------------------
            win_tiles = {}

            def ensure_window(w):
                if w in win_tiles:
                    return win_tiles[w]
                c0 = w * CW
                c1 = min((w + 1) * CW, NCOL)
                ncw = c1 - c0
                G = gp.tile([P, CW * H], dt.bfloat16, tag="g")
                nc.sync.dma_start(G[:, : ncw * H],
                                  et_d[:, c0 * H: c1 * H])
                ohs = op_.tile([P, CW, P], dt.bfloat16, tag="oh")
                for c in range(c0, c1):
                    b = int(occ_block[c])
                    nc.vector.scalar_tensor_tensor(
                        out=ohs[:, c - c0, :],
                        in0=iota_f[:],
                        scalar=dstl_t[:, c: c + 1],
                        in1=recipR[:, b * P: (b + 1) * P],
                        op0=mybir.AluOpType.is_equal,
                        op1=mybir.AluOpType.mult)
                win_tiles[w] = (G, ohs, c0)
                return win_tiles[w]

            MB = 4                     # blocks per MLP chunk (1 PSUM bank)
            OB = 28                    # blocks per output-staging DMA
            meanT_stage = None
            stage_out = None
            for b in range(BPC):
                mj = b % MB
                nmb = min(MB, BPC - (b - mj))
                sj = b % OB
                if mj == 0:
                    meanT_stage = mp.tile([P, MB * P], dt.bfloat16,
                                          tag="meanT")
                if sj == 0:
                    stage_out = stp.tile([P, OB * P], dt.bfloat16, tag="st")

                meanT_ps = sums_pp.tile([P, P], dt.float32, space="PSUM",
                                        tag="sums")
                nmm = int(mcols[b])
                for i in range(nmm):
                    c = int(colbase[b]) + i
                    G, ohs, c0 = ensure_window(c // CW)
                    nc.tensor.matmul(out=meanT_ps[:],
                                     lhsT=G[:, (c - c0) * H: (c - c0 + 1) * H],
                                     rhs=ohs[:, c - c0, :],
                                     start=(i == 0),
                                     stop=(i == nmm - 1))
                nc.scalar.activation(meanT_stage[:, mj * P: (mj + 1) * P],
                                     meanT_ps[:],
                                     mybir.ActivationFunctionType.Copy)

                if mj == nmb - 1:
                    bm = b - mj
                    hpre_ps = hpre_pp.tile([P, MB * H], dt.float32,
                                           space="PSUM", tag="hpre")
                    nc.tensor.matmul(out=hpre_ps[:, : nmb * H], lhsT=w1_t[:],
                                     rhs=meanT_stage[:, : nmb * P],
                                     start=True, stop=True)
                    hT_stage = mp.tile([P, MB * H], dt.bfloat16, tag="hT")
                    nc.scalar.activation(hT_stage[:, : nmb * H],
                                         hpre_ps[:, : nmb * H],
                                         mybir.ActivationFunctionType.Gelu,
                                         bias=b1_t[:, :1])
                    rep_ps = rep_pp.tile([P, MB * P], dt.float32,
                                         space="PSUM", tag="rep")
                    nc.tensor.matmul(out=rep_ps[:, : nmb * P], lhsT=w2_t[:],
                                     rhs=hT_stage[:, : nmb * H],
                                     start=True, stop=True)
                    nc.scalar.activation(
                        stage_out[:, (bm % OB) * P: (bm % OB + nmb) * P],
                        rep_ps[:, : nmb * P],
                        mybir.ActivationFunctionType.Identity,
                        bias=b2_t[:, :1])
                if sj == OB - 1 or b == BPC - 1:
                    b0 = b - sj
                    nc.sync.dma_start(
                        reports_o[:, b0 * P: (b + 1) * P],
                        stage_out[:, : (sj + 1) * P])
    nc.compile()
    return nc


# ---------------------------------------------------------------------------
# launch 2: leader election + output gather (reports bf16)
# ---------------------------------------------------------------------------

def _build_l2(pp, batched_indirect):
    WU = pp["WU"]
    SW = BPC * WU
    nc = bacc.Bacc("TRN2", target_bir_lowering=False, debug=False,
                   num_devices=NCORES)
    ep_d = nc.dram_tensor("epad", [P, SW], dt.float32, kind="ExternalInput")
    sp1_d = nc.dram_tensor("srcp1", [P, SW], dt.float32,
                           kind="ExternalInput")
    rep_d = nc.dram_tensor("repfull", [NPAD, C], dt.bfloat16,
                           kind="ExternalInput")
    out_o = nc.dram_tensor("gath_o", [P, BPC * C], dt.bfloat16,
                           kind="ExternalOutput")

    GB = 14  # blocks per gather/output chunk

    with tile.TileContext(nc) as tc:
        with (
            tc.tile_pool(name="sb", bufs=1) as sb,
            tc.tile_pool(name="rows", bufs=2) as rp_,
        ):
            ep = sb.tile([P, BPC, WU], dt.float32)
            nc.sync.dma_start(
                ep[:], ep_d[:, :].rearrange("p (b w) -> p b w", w=WU))
            sp1 = sb.tile([P, BPC, WU], dt.float32)
            nc.sync.dma_start(
                sp1[:], sp1_d[:, :].rearrange("p (b w) -> p b w", w=WU))

            sm = sb.tile([P, BPC], dt.float32)
            nc.vector.reduce_max(out=sm[:], in_=ep[:],
                                 axis=mybir.AxisListType.X)
            mask = sb.tile([P, BPC, WU], dt.float32)
            sm_b = bass.AP(sm.tensor, 0, [sm[:].ap[0], [1, BPC], [0, WU]])
            nc.vector.tensor_tensor(out=mask[:], in0=ep[:], in1=sm_b,
                                    op=mybir.AluOpType.is_equal)
            cand = sb.tile([P, BPC, WU], dt.float32)
            nc.vector.tensor_tensor(out=cand[:], in0=mask[:], in1=sp1[:],
                                    op=mybir.AluOpType.mult)
            lp1 = sb.tile([P, BPC], dt.float32)
            nc.vector.reduce_max(out=lp1[:], in_=cand[:],
                                 axis=mybir.AxisListType.X)
            leadf = sb.tile([P, BPC], dt.float32)
            nc.vector.tensor_scalar(
                out=leadf[:], in0=lp1[:], scalar1=-1.0, scalar2=0.0,
                op0=mybir.AluOpType.add, op1=mybir.AluOpType.max)
            leadi = sb.tile([P, BPC], dt.int32)
            nc.vector.tensor_copy(leadi[:], leadf[:])

            for g0 in range(0, BPC, GB):
                ng = min(GB, BPC - g0)
                rows = rp_.tile([P, GB, C], dt.bfloat16, tag="rows")
                if batched_indirect:
                    nc.gpsimd.indirect_dma_start(
                        out=rows[:, :ng, :],
                        out_offset=None,
                        in_=rep_d[:, :],
                        in_offset=bass.IndirectOffsetOnAxis(
                            ap=leadi[:, g0: g0 + ng], axis=0),
                    )
                else:
                    for j in range(ng):
                        nc.gpsimd.indirect_dma_start(
                            out=rows[:, j, :],
                            out_offset=None,
                            in_=rep_d[:, :],
                            in_offset=bass.IndirectOffsetOnAxis(
                                ap=leadi[:, g0 + j: g0 + j + 1], axis=0),
                        )
                nc.sync.dma_start(
                    out_o[:, g0 * C: (g0 + ng) * C].rearrange(
                        "p (g c) -> p g c", c=C),
                    rows[:, :ng, :])
    nc.compile()
    return nc


# ---------------------------------------------------------------------------

_CACHE = {}


def _get(key, fn):
    if key not in _CACHE:
        _CACHE[key] = fn()
    return _CACHE[key]


BATCHED_INDIRECT = False


def kernel(x, edge_index, w_lead, b_lead, w1, b1, w2, b2):
    x = np.asarray(x, np.float32)
    assert x.shape == (N_NODES, H)

    ekey = hashlib.md5(np.asarray(edge_index).tobytes()).hexdigest()
    pp = _get(("pp", ekey), lambda: _preprocess(edge_index))

    NCOL = pp["NCOL"]
    inv = pp["inv"]
    inv_flat = inv.reshape(NCORES, NPC)

    xbf_ext = np.zeros((N_NODES + 1, H), bf16)
    xbf_ext[:N_NODES] = x.astype(bf16)
    xpad = np.zeros((NPAD, H), np.float32)
    xpad[:N_NODES] = x

    wrep = np.tile(np.asarray(w_lead, np.float32)[None, :], (P, 1))
    blead = np.full((P, 1), np.float32(b_lead), np.float32)
    w1f = np.ascontiguousarray(np.asarray(w1, np.float32).astype(bf16))
    b1c = np.ascontiguousarray(np.asarray(b1, np.float32).reshape(H, 1))
    w2f = np.ascontiguousarray(np.asarray(w2, np.float32).astype(bf16))
    b2c = np.ascontiguousarray(np.asarray(b2, np.float32).reshape(C, 1))

    # ---- launch 1 ----------------------------------------------------------
    nc1 = _get(("l1", ekey), lambda: _build_l1(pp))
    in_maps = []
    for k in range(NCORES):
        # ET[k]: [P, NCOL*H] partition-major slot table (slot = c*128+p)
        et = xbf_ext[pp["srcidx"][k]].reshape(NCOL, P, H).transpose(1, 0, 2)
        et = np.ascontiguousarray(et).reshape(P, NCOL * H)
        # xf2: [P, BPC*H] partition-major owned-node features
        xf2 = np.where((inv[k] >= 0)[:, :, None],
                       xpad[np.maximum(inv[k], 0)], 0.0)  # [BPC, P, H]
        xf2 = np.ascontiguousarray(
            xf2.transpose(1, 0, 2).reshape(P, BPC * H)).astype(np.float32)
        in_maps.append({
            "et": et,
            "xf2": xf2,
            "dstl": pp["dstl_bf"][k],
            "recipf": pp["recip_flat"][k],
            "wrep": wrep,
            "blead": blead,
            "w1": w1f,
            "b1": b1c,
            "w2": w2f,
            "b2c": b2c,
        })
    r1 = run_bass_kernel_spmd(nc1, in_maps, core_ids=CORES)

    logits_full = np.zeros(NPAD, np.float32)
    reports_full = np.zeros((NPAD, C), bf16)
    for k in range(NCORES):
        lg = r1.results[k]["logits_o"]                  # [P, BPC]
        rp = r1.results[k]["reports_o"].reshape(C, BPC, P)
        m = inv_flat[k] >= 0
        ids = inv_flat[k][m]
        logits_full[ids] = lg.T.reshape(-1)[m]
        reports_full[ids] = rp.transpose(1, 2, 0).reshape(NPC, C)[m]

    # ---- launch 2: leader election + gather --------------------------------
    nc2 = _get(("l2", ekey), lambda: _build_l2(pp, BATCHED_INDIRECT))
    es = pp["elog_src"]
    in_maps2 = []
    for k in range(NCORES):
        ep = np.where(es[k] >= 0, logits_full[np.maximum(es[k], 0)], NEG)
        in_maps2.append({
            "epad": np.ascontiguousarray(ep.astype(np.float32)),
            "srcp1": pp["srcp1"][k],
            "repfull": reports_full,
        })
    r2 = run_bass_kernel_spmd(nc2, in_maps2, core_ids=CORES)

    out = np.zeros((N_NODES, C), np.float32)
    for k in range(NCORES):
        g = r2.results[k]["gath_o"].reshape(P, BPC, C).astype(np.float32)
        node_rows = g.transpose(1, 0, 2).reshape(NPC, C)
        m = inv_flat[k] >= 0
        out[inv_flat[k][m]] = node_rows[m]
    return out
